# revision 2
# baseline (speedup 1.0000x reference)
"""Causal self-attention TRN2 kernel, v2: bf16 + transposed AV.

Full inputs in, full output out. Core c = 4*b + g runs batch b (of 2) and
head-group g (4 of 16 heads). Host pre-transposes and casts to bf16:

  xT  [1024, 2048] = x[b].T
  wqT/wkT/wvT [1024, 256] = w[rows of group].T   (wq pre-scaled by 1/8)
  woT [256, 1024] = wo[:, cols of group].T

Differences vs v1:
- all matmul inputs bf16 (PSUM stays f32): removes the fp32r ap<256 4x
  penalty on diagonal tiles, halves DMA and DVE elementwise costs
- AV is transposed: stationary = exp(S^T) chunk [128 keys, 128 q],
  moving = v [128 keys, 65] (col 64 = ones -> denominator lands in
  column 64 of the output). out = attnout natural [q, 65] per head.
  65-cycle moving passes instead of 512: AV drops ~2x in PE cycles and
  the K=1 broadcast matmuls of v1 disappear (denominator is now
  per-PARTITION, handled by one DVE multiply with a free-dim broadcast)
- attnout natural -> PE transpose (128x128 blocks) to aT for the
  output projection
- schedule: all q/k/v projections first, then scores/AV windows
  interleaved one window apart so ACT exp overlaps PE AV/outproj work
"""

from contextlib import ExitStack

import numpy as np
import ml_dtypes

from concourse import bacc, bass, mybir, tile
from concourse.bass_utils import run_bass_kernel_spmd
from concourse.masks import make_upper_triangular, make_identity

B, T, D = 2, 2048, 1024
H, DH = 16, 64
N_CORES = 8
HG = 4                # tensor-parallel groups
HPG = H // HG         # heads per group = 4
CL = HPG * DH         # local channels = 256
KC = D // 128         # contraction chunks over D = 8
TQ = T // 512         # 512-wide T windows = 4
F32 = mybir.dt.float32
F32R = mybir.dt.float32r
BF16 = mybir.dt.bfloat16


class Ctx:
    pass


def geom(w, ki):
    """Width and q-offset of score chunk ki in window w."""
    if ki < 4 * w:
        return 512, 0
    wd = 512 - 128 * (ki - 4 * w)
    return wd, 512 - wd


def emit_consts(ctx, tc, g):
    nc = tc.nc
    persist = ctx.enter_context(tc.tile_pool(name="persist", bufs=1))
    g.xt_pool = ctx.enter_context(tc.tile_pool(name="xt", bufs=2))
    g.etF_pool = ctx.enter_context(tc.tile_pool(name="etF", bufs=40))
    g.etB_pool = ctx.enter_context(tc.tile_pool(name="etB", bufs=9))
    g.etC_pool = ctx.enter_context(tc.tile_pool(name="etC", bufs=9))
    g.ysb_pool = ctx.enter_context(tc.tile_pool(name="ysb", bufs=3))
    g.rc_pool = ctx.enter_context(tc.tile_pool(name="rc", bufs=4))
    g.asb_pool = ctx.enter_context(tc.tile_pool(name="asb", bufs=3))
    # PSUM, 8 banks: st 2x2 + pj 2x1 + av 2x1
    g.pp = ctx.enter_context(tc.tile_pool(name="pp", bufs=2, space="PSUM"))

    g.mask01 = persist.tile([128, 128], BF16, tag="mask01", name="mask01")
    make_upper_triangular(nc, g.mask01[:, :], val=1.0, diag=True)
    g.ident = persist.tile([128, 128], BF16, tag="ident", name="ident")
    make_identity(nc, g.ident[:, :])

    ones_f32 = persist.tile([128, HPG], F32, tag="ones_f32", name="ones_f32")
    nc.vector.memset(ones_f32[:, :], 1.0)

    g.wq_all = persist.tile([128, KC * CL], F32R, tag="wq_all", name="wq_all")
    g.wk_all = persist.tile([128, KC * CL], F32R, tag="wk_all", name="wk_all")
    g.wv_all = persist.tile([128, KC * CL], F32R, tag="wv_all", name="wv_all")
    g.wo_all = persist.tile([128, 2 * D], BF16, tag="wo_all", name="wo_all")
    g.wq_sb = [g.wq_all[:, CL * i:CL * i + CL] for i in range(KC)]
    g.wk_sb = [g.wk_all[:, CL * i:CL * i + CL] for i in range(KC)]
    g.wv_sb = [g.wv_all[:, CL * i:CL * i + CL] for i in range(KC)]
    g.wo_sb = [g.wo_all[:, D * i:D * i + D] for i in range(2)]

    g.qT_sb = [persist.tile([128, T], BF16, tag=f"qT{i}", name=f"qT{i}") for i in range(2)]
    g.kT_sb = [persist.tile([128, T], BF16, tag=f"kT{i}", name=f"kT{i}") for i in range(2)]
    g.aT_sb = [persist.tile([128, T], BF16, tag=f"aT{i}", name=f"aT{i}") for i in range(2)]

    # v natural layout, one tile per 128-row k-chunk, head-strided cols of 65
    # (col 65h+64 is the ones column -> AV output col 64 = softmax denom)
    g.v_sb = [persist.tile([128, HPG * 65], BF16, tag=f"v{i}", name=f"v{i}")
              for i in range(T // 128)]
    for i in range(T // 128):
        ones_cols = g.v_sb[i].rearrange("p (h c) -> p h c", c=65)[:, :, 64:65]
        nc.vector.tensor_copy(ones_cols, ones_f32.rearrange("p (h c) -> p h c", c=1))


def emit_qkvproj(tc, g, xT, w, wqT=None, wkT=None, wvT=None, woT=None):
    nc = tc.nc
    ts = 512 * w
    xt_all = g.xt_pool.tile([128, KC * 512], F32R, tag="xt", name="xt")
    for half in range(2):
        nc.sync.dma_start(
            out=xt_all.rearrange("p (kc t) -> p kc t", t=512)[:, 4 * half:4 * half + 4],
            in_=xT.rearrange("(kc p) t -> p kc t", p=128)[:, 4 * half:4 * half + 4,
                                                          ts:ts + 512],
        )
    xt = [xt_all[:, 512 * kc:512 * kc + 512] for kc in range(KC)]

    for (w_sb, dst, wT, w_all, ev_eng) in (
            (g.wq_sb, g.qT_sb, wqT, g.wq_all, nc.vector),
            (g.wk_sb, g.kT_sb, wkT, g.wk_all, nc.vector)):
        if wT is not None:
            nc.scalar.dma_start(
                out=w_all.rearrange("p (kc c) -> p kc c", c=CL),
                in_=wT.rearrange("(kc p) c -> p kc c", p=128),
            )
        for m in range(2):
            ps = g.pp.tile([128, 512], F32, tag="pj", bufs=2, name="psqk")
            for kc in range(KC):
                nc.tensor.matmul(
                    out=ps[:, :],
                    lhsT=w_sb[kc][:, 128 * m:128 * m + 128],
                    rhs=xt[kc][:, :],
                    start=(kc == 0),
                    stop=(kc == KC - 1),
                )
            ev_eng.tensor_copy(dst[m][:, ts:ts + 512], ps[:, :])

    if wvT is not None:
        nc.scalar.dma_start(
            out=g.wv_all.rearrange("p (kc c) -> p kc c", c=CL),
            in_=wvT.rearrange("(kc p) c -> p kc c", p=128),
        )
    for tc4 in range(4):
        tg = 4 * w + tc4
        ps = g.pp.tile([128, CL], F32, tag="pj", bufs=2, name="psv")
        for kc in range(KC):
            nc.tensor.matmul(
                out=ps[:, :],
                lhsT=xt[kc][:, 128 * tc4:128 * tc4 + 128],
                rhs=g.wv_sb[kc][:, :],
                start=(kc == 0),
                stop=(kc == KC - 1),
            )
        nc.vector.tensor_copy(
            g.v_sb[tg].rearrange("p (h c) -> p h c", c=65)[:, :, 0:64],
            ps.rearrange("p (h c) -> p h c", c=64)[:, :, :],
        )
    if woT is not None:
        nc.scalar.dma_start(
            out=g.wo_all.rearrange("p (cc d) -> p cc d", d=D),
            in_=woT.rearrange("(cc p) d -> p cc d", p=128),
        )


def emit_scores(tc, g, w, ets):
    """Score matmuls + exp + diagonal masking for window w.

    Fills ets[(hp, pi, hh)] = (et_tile, w0, qoff0, w1, qoff1)."""
    nc = tc.nc
    qs = 512 * w
    nk = 4 * w + 4
    for hp in range(2):
        for pi in range(nk // 2):
            ki0, ki1 = 2 * pi, 2 * pi + 1
            w0, qoff0 = geom(w, ki0)
            w1, qoff1 = geom(w, ki1)
            wid = w0 + w1
            for hh in range(2):
                po = 64 * hh
                st = g.pp.tile([128, 1024], F32, tag="st", bufs=2, name="st")
                for (ki, wd, qoff, co) in ((ki0, w0, qoff0, 0), (ki1, w1, qoff1, w0)):
                    nc.tensor.matmul(
                        out=st[:, co:co + wd],
                        lhsT=g.kT_sb[hp][po:po + 64, 128 * ki:128 * ki + 128],
                        rhs=g.qT_sb[hp][po:po + 64, qs + qoff:qs + 512],
                        start=True,
                        stop=True,
                    )
                if wid == 1024:
                    et = g.etF_pool.tile([128, 1024], BF16, tag="etF", name="et")
                elif wid == 896:
                    et = g.etB_pool.tile([128, 896], BF16, tag="etB", name="et")
                else:
                    et = g.etC_pool.tile([128, 384], BF16, tag="etC", name="et")
                nc.scalar.activation(
                    out=et[:, :wid],
                    in_=st[:, :wid],
                    func=mybir.ActivationFunctionType.Exp,
                )
                if ki0 >= 4 * w:  # diagonal 128x128 block
                    nc.vector.tensor_mul(et[:, 0:128], et[:, 0:128], g.mask01[:, :])
                if ki1 >= 4 * w:
                    nc.vector.tensor_mul(et[:, w0:w0 + 128], et[:, w0:w0 + 128],
                                         g.mask01[:, :])
                ets[(hp, pi, hh)] = (et, w0, qoff0, w1, qoff1)


def emit_av(tc, g, w, ets):
    """Transposed AV + normalize + transpose-to-aT for window w."""
    nc = tc.nc
    for qb in range(4):
        gqb = 4 * w + qb  # global q block; chunks 0..gqb reach it
        av = g.pp.tile([128, HPG * 65], F32, tag="av", bufs=2, name="av")
        avh = av.rearrange("p (h c) -> p h c", c=65)
        for kc in range(gqb + 1):
            pi, slot = divmod(kc, 2)
            for h in range(HPG):
                hp, hh = divmod(h, 2)
                et, w0, qoff0, w1, qoff1 = ets[(hp, pi, hh)]
                co = 0 if slot == 0 else w0
                qoff = qoff0 if slot == 0 else qoff1
                col = co + 128 * qb - qoff
                nc.tensor.matmul(
                    out=av[:, 65 * h:65 * h + 65],
                    lhsT=et[:, col:col + 128],
                    rhs=g.v_sb[kc][:, 65 * h:65 * h + 65],
                    start=(kc == 0 and h == 0),
                    stop=(kc == gqb and h == HPG - 1),
                    skip_group_check=True,
                )
        # denominators sit in column 64 of each head's 65-block
        rc = g.rc_pool.tile([128, HPG], F32, tag="rc", name="rc")
        with nc.allow_low_precision(reason="softmax denominator"):
            nc.vector.reciprocal(
                rc.rearrange("p (h c) -> p h c", c=1), avh[:, :, 64:65])
        a_sb = g.asb_pool.tile([128, CL], BF16, tag="asb", name="asb")
        nc.vector.tensor_mul(
            a_sb.rearrange("p (h c) -> p h c", c=64),
            avh[:, :, 0:64],
            rc.rearrange("p (h c) -> p h c", c=1).broadcast_to([128, HPG, 64]),
        )
        for cc in range(2):
            at_ps = g.pp.tile([128, 128], BF16, tag="pj", bufs=2, name="atps")
            nc.tensor.transpose(
                at_ps[:, :], a_sb[:, 128 * cc:128 * cc + 128], g.ident[:, :])
            nc.vector.tensor_copy(
                g.aT_sb[cc][:, 128 * gqb:128 * gqb + 128], at_ps[:, :])


def emit_outproj(tc, g, y, w):
    nc = tc.nc
    for tc4 in range(4):
        tg = 4 * w + tc4
        ysb = g.ysb_pool.tile([128, D], BF16, tag="ysb", name="ysb")
        for dj in range(2):
            py = g.pp.tile([128, 512], F32, tag="pj", bufs=2, name="py")
            for cc in range(2):
                nc.tensor.matmul(
                    out=py[:, :],
                    lhsT=g.aT_sb[cc][:, 128 * tg:128 * tg + 128],
                    rhs=g.wo_sb[cc][:, 512 * dj:512 * dj + 512],
                    start=(cc == 0),
                    stop=(cc == 1),
                )
            nc.scalar.activation(
                out=ysb[:, 512 * dj:512 * dj + 512], in_=py[:, :],
                func=mybir.ActivationFunctionType.Copy)
        nc.sync.dma_start(out=y[128 * tg:128 * tg + 128, :], in_=ysb[:, :])


def attn_kernel(ctx, tc, y, xT, wqT, wkT, wvT, woT, n_reps=1):
    g = Ctx()
    emit_consts(ctx, tc, g)
    for rep in range(n_reps):
        first = rep == 0
        for w in range(TQ):
            emit_qkvproj(tc, g, xT, w,
                         wqT=wqT if first and w == 0 else None,
                         wkT=wkT if first and w == 0 else None,
                         wvT=wvT if first and w == 0 else None,
                         woT=woT if first and w == 0 else None)
        all_ets = [dict() for _ in range(TQ)]
        emit_scores(tc, g, 0, all_ets[0])
        emit_scores(tc, g, 1, all_ets[1])
        emit_av(tc, g, 0, all_ets[0])
        emit_outproj(tc, g, y, 0)
        emit_scores(tc, g, 2, all_ets[2])
        emit_av(tc, g, 1, all_ets[1])
        emit_outproj(tc, g, y, 1)
        emit_scores(tc, g, 3, all_ets[3])
        emit_av(tc, g, 2, all_ets[2])
        emit_outproj(tc, g, y, 2)
        emit_av(tc, g, 3, all_ets[3])
        emit_outproj(tc, g, y, 3)


_PROGRAMS = {}


def get_program(n_reps=1):
    key = n_reps
    if key not in _PROGRAMS:
        nc = bacc.Bacc("TRN2", target_bir_lowering=False, debug=False,
                       num_devices=N_CORES)
        xT = nc.dram_tensor("xT", [D, T], F32R, kind="ExternalInput").ap()
        wqT = nc.dram_tensor("wqT", [D, CL], F32R, kind="ExternalInput").ap()
        wkT = nc.dram_tensor("wkT", [D, CL], F32R, kind="ExternalInput").ap()
        wvT = nc.dram_tensor("wvT", [D, CL], F32R, kind="ExternalInput").ap()
        woT = nc.dram_tensor("woT", [CL, D], BF16, kind="ExternalInput").ap()
        y = nc.dram_tensor("y", [T, D], BF16, kind="ExternalOutput").ap()
        with tile.TileContext(nc) as tc:
            with ExitStack() as ctx:
                attn_kernel(ctx, tc, y, xT, wqT, wkT, wvT, woT, n_reps=n_reps)
        nc.compile()
        _PROGRAMS[key] = nc
    return _PROGRAMS[key]


def make_in_maps(x, wq, wk, wv, wo):
    x = np.asarray(x, np.float32)
    wq, wk, wv, wo = (np.asarray(a, np.float32) for a in (wq, wk, wv, wo))
    scale = np.float32(DH ** -0.5)
    bf = ml_dtypes.bfloat16
    in_maps = []
    for c in range(N_CORES):
        b, gi = divmod(c, HG)
        rows = slice(gi * CL, (gi + 1) * CL)
        in_maps.append({
            "xT": np.ascontiguousarray(x[b].T),
            "wqT": np.ascontiguousarray(wq[rows].T) * scale,
            "wkT": np.ascontiguousarray(wk[rows].T),
            "wvT": np.ascontiguousarray(wv[rows].T),
            "woT": np.ascontiguousarray(wo[:, rows].T).astype(bf),
        })
    return in_maps


def gather(results):
    y = np.zeros((B, T, D), np.float32)
    for c in range(N_CORES):
        y[c // HG] += np.asarray(results[c]["y"], np.float32)
    return y


def kernel(x, wq, wk, wv, wo):
    nc = get_program()
    in_maps = make_in_maps(x, wq, wk, wv, wo)
    res = run_bass_kernel_spmd(nc, in_maps, list(range(N_CORES)))
    return gather(res.results)
